# revision 3
# baseline (speedup 1.0000x reference)
"""Trainium2 kernel for nn_Neighborlist (fused gather->diff->norm->screen).

Sharding: pairs are split evenly across the 8 NeuronCores (data parallel, per
the sharding hint). The host prepares per-core streams; each core computes
diff = cA - cB + shift, dist, and the cutoff/dummy screen, and writes the
masked outputs. The dummy-atom (species == -1) mask is folded into the
coordinate tables on the host: dummy atoms get a large per-atom offset (+ for
the i0 side, - for the i1 side) so that any pair involving a dummy atom
screens out via the cutoff test; non-dummy coordinates are bit-identical to
the originals so kept pairs are exact.

Note on the gather: the intended on-device gather from a replicated coords
table is not expressible on this toolchain — indirect_dma_start consumes only
one index per partition on HW, InstDMAGatherAnt (ext-isa dma_gather) wedges
the device (NRT_EXEC_UNIT_UNRECOVERABLE), and XLA's own gather lowering
crashes neuronx-cc on this module. The gather is therefore done host-side as
part of input sharding; the full screen/norm pipeline runs on device.

The cutoff test is done on squared distance against a precomputed threshold
DSQ_MAX = max{x : sqrt_f32(x) <= f32(5.2)}, which matches the reference's
sqrt-then-compare bit-exactly. dist itself is ACT Sqrt refined with one
Newton-Raphson step (rsqrt form) on the vector engine.
"""
import numpy as np

import concourse.bass as bass
import concourse.mybir as mybir
from concourse import bass_utils
from concourse.tile import TileContext
from concourse.vector_clock import ScopedClock
from concourse.alu_op_type import AluOpType

CUTOFF = np.float32(5.2)
N_CORES = 8
P_TOTAL = 6_400_000
P_CORE = P_TOTAL // N_CORES          # 800_000
ROWS = 128
COLS = P_CORE // ROWS                # 6250
F = 625                              # pairs per partition-row per tile
N_TILES = COLS // F                  # 10

_MODE_TO_OP = {
    "sem-ge-imm": "sem-ge",
    "sem-eq-imm": "sem-eq",
    "sem-le-imm": "sem-le",
    "sem-gt-imm": "sem-gt",
    "sem-lt-imm": "sem-lt",
}


def _patched_drain_and_barrier(self, tick_clock, wait_clock):
    # This walrus build encodes at most one sync wait per instruction and none
    # on TPB_CTRL (Drain/NoOp); emit the tail-drain waits as standalone
    # EventSemaphore instructions instead.
    import bass_rust
    nc = self.nc
    carrier = nc.sync.nop(nofuse=True, hint="pre_drain_waits")
    wait_clock.add_sem_waits(
        carrier.ins, ScopedClock({None: tick_clock.global_clock})
    )
    si = carrier.ins.sync_info
    waits = list(si.on_wait) if si is not None and si.on_wait else []
    if waits:
        si.on_wait = []
        for w in waits:
            handle = bass_rust.SemaphoreHandle(name=w.ant_name, num=w.id)
            nc.sync.wait_op(handle, w.wait_value, _MODE_TO_OP[w.wait_mode])
    nc.sync.drain()
    nc.all_engine_barrier()
    assert self.sems is not None
    popped = nc._tile_sem_poison_stack.pop()
    assert popped is self._sem_poison
    nc.clear_and_free_semaphores(list(self.sems.allocated().values()))
    nc.all_engine_barrier()


def _split_multi_waits(nc):
    # Hoist extra sync waits (>1 per instruction) into standalone
    # EventSemaphore instructions on the same engine.
    import bass_rust
    n = 0
    for f in nc.m.functions:
        for bb in f.blocks:
            out = []
            for inst in list(bb.instructions):
                si = inst.sync_info
                if si is not None and si.on_wait and len(si.on_wait) > 1:
                    extra = list(si.on_wait[:-1])
                    si.on_wait = si.on_wait[-1:]
                    for w in extra:
                        n += 1
                        ev = bass_rust.InstEventSemaphore(
                            name=f"I-msw-{n}-{inst.name}",
                            opcode="EventSemaphore",
                            engine=inst.engine,
                            debug=inst.debug,
                            ins=[],
                            outs=[],
                            sync_info=bass_rust.SyncInfo(on_wait=[w], on_update=[]),
                        )
                        out.append(ev)
                out.append(inst)
            bb.instructions = out
    return n


def _dsq_max():
    """Largest f32 x with sqrt_f32(x) <= f32(5.2)."""
    c = np.float32(5.2)
    x = np.float32(c * c)
    while np.sqrt(np.float32(x)) <= c:
        x = np.nextafter(x, np.float32(np.inf), dtype=np.float32)
    while np.sqrt(np.float32(x)) > c:
        x = np.nextafter(x, np.float32(-np.inf), dtype=np.float32)
    return float(x)


_NC_CACHE = {}


def _build_kernel():
    if "nc" in _NC_CACHE:
        return _NC_CACHE["nc"]
    from concourse.tile import TileContext as TC
    TC._drain_and_barrier = _patched_drain_and_barrier

    dsq_max = _dsq_max()
    f32 = mybir.dt.float32
    i32 = mybir.dt.int32
    nc = bass.Bass()
    ca = nc.dram_tensor("ca", [ROWS, COLS * 3], f32, kind="ExternalInput")
    cb = nc.dram_tensor("cb", [ROWS, COLS * 3], f32, kind="ExternalInput")
    sh = nc.dram_tensor("sh", [ROWS, COLS * 3], f32, kind="ExternalInput")
    i0 = nc.dram_tensor("i0", [ROWS, COLS], i32, kind="ExternalInput")
    i1 = nc.dram_tensor("i1", [ROWS, COLS], i32, kind="ExternalInput")
    o_i0 = nc.dram_tensor("o_i0", [ROWS, COLS], i32, kind="ExternalOutput")
    o_i1 = nc.dram_tensor("o_i1", [ROWS, COLS], i32, kind="ExternalOutput")
    o_d = nc.dram_tensor("o_d", [ROWS, COLS], f32, kind="ExternalOutput")
    o_v = nc.dram_tensor("o_v", [ROWS, COLS * 3], f32, kind="ExternalOutput")

    with TileContext(nc) as tc:
        with tc.tile_pool(name="p", bufs=2) as pool:
            for t in range(N_TILES):
                s3 = slice(t * F * 3, (t + 1) * F * 3)
                s1 = slice(t * F, (t + 1) * F)
                ca_t = pool.tile([ROWS, F * 3], f32, tag="ca")
                cb_t = pool.tile([ROWS, F * 3], f32, tag="cb")
                sh_t = pool.tile([ROWS, F * 3], f32, tag="sh")
                i0_t = pool.tile([ROWS, F], i32, tag="i0")
                i1_t = pool.tile([ROWS, F], i32, tag="i1")
                nc.sync.dma_start(ca_t[:], ca[:, s3])
                nc.sync.dma_start(cb_t[:], cb[:, s3])
                nc.sync.dma_start(sh_t[:], sh[:, s3])
                nc.sync.dma_start(i0_t[:], i0[:, s1])
                nc.sync.dma_start(i1_t[:], i1[:, s1])

                d3 = pool.tile([ROWS, F * 3], f32, tag="d3")
                nc.vector.tensor_tensor(
                    out=d3[:], in0=ca_t[:], in1=cb_t[:], op=AluOpType.subtract)
                nc.vector.tensor_tensor(
                    out=d3[:], in0=d3[:], in1=sh_t[:], op=AluOpType.add)
                sq = pool.tile([ROWS, F * 3], f32, tag="sq")
                nc.vector.tensor_tensor(
                    out=sq[:], in0=d3[:], in1=d3[:], op=AluOpType.mult)
                dsq = pool.tile([ROWS, F], f32, tag="dsq")
                nc.vector.reduce_sum(
                    dsq[:], sq[:].rearrange("p (k c) -> p k c", c=3),
                    axis=mybir.AxisListType.X)

                # dist = sqrt(dsq) : ACT Sqrt + one Newton step via exact DVE
                # reciprocal; dist=0 at dsq=0.
                d0 = pool.tile([ROWS, F], f32, tag="d0")
                nc.scalar.activation(
                    d0[:], dsq[:], mybir.ActivationFunctionType.Sqrt)
                dg = pool.tile([ROWS, F], f32, tag="dg")
                nc.vector.tensor_scalar(
                    out=dg[:], in0=d0[:], scalar1=1e-30, scalar2=None,
                    op0=AluOpType.max)
                inv = pool.tile([ROWS, F], f32, tag="inv")
                nc.vector.reciprocal(out=inv[:], in_=dg[:])
                y0 = pool.tile([ROWS, F], f32, tag="y0")
                nc.vector.tensor_tensor(
                    out=y0[:], in0=dsq[:], in1=inv[:], op=AluOpType.mult)
                dist = pool.tile([ROWS, F], f32, tag="dist")
                nc.vector.tensor_tensor(
                    out=dist[:], in0=d0[:], in1=y0[:], op=AluOpType.add)
                nc.vector.tensor_scalar(
                    out=dist[:], in0=dist[:], scalar1=0.5, scalar2=None,
                    op0=AluOpType.mult)

                keep = pool.tile([ROWS, F], f32, tag="keep")
                nc.vector.tensor_scalar(
                    out=keep[:], in0=dsq[:], scalar1=dsq_max, scalar2=None,
                    op0=AluOpType.is_le)

                od_t = pool.tile([ROWS, F], f32, tag="od")
                nc.vector.tensor_tensor(
                    out=od_t[:], in0=dist[:], in1=keep[:], op=AluOpType.mult)
                nc.sync.dma_start(o_d[:, s1], od_t[:])

                ov_t = pool.tile([ROWS, F * 3], f32, tag="ov")
                d3v = d3[:].rearrange("p (k c) -> p k c", c=3)
                ovv = ov_t[:].rearrange("p (k c) -> p k c", c=3)
                for c in range(3):
                    nc.vector.tensor_tensor(
                        out=ovv[:, :, c], in0=d3v[:, :, c], in1=keep[:],
                        op=AluOpType.mult)
                nc.sync.dma_start(o_v[:, s3], ov_t[:])

                for idx_t, o_t in ((i0_t, o_i0), (i1_t, o_i1)):
                    f_t = pool.tile([ROWS, F], f32, tag="fidx")
                    nc.vector.tensor_copy(out=f_t[:], in_=idx_t[:])
                    nc.vector.tensor_scalar(
                        out=f_t[:], in0=f_t[:], scalar1=1.0, scalar2=None,
                        op0=AluOpType.add)
                    nc.vector.tensor_tensor(
                        out=f_t[:], in0=f_t[:], in1=keep[:], op=AluOpType.mult)
                    nc.vector.tensor_scalar(
                        out=f_t[:], in0=f_t[:], scalar1=-1.0, scalar2=None,
                        op0=AluOpType.add)
                    oi_t = pool.tile([ROWS, F], i32, tag="oidx")
                    nc.vector.tensor_copy(out=oi_t[:], in_=f_t[:])
                    nc.sync.dma_start(o_t[:, s1], oi_t[:])

    _split_multi_waits(nc)
    _NC_CACHE["nc"] = nc
    return nc


def kernel(species, coordinates, neighbor_idxs, shift_values):
    species = np.asarray(species)
    coordinates = np.asarray(coordinates)
    neighbor_idxs = np.asarray(neighbor_idxs)
    shift_values = np.asarray(shift_values)

    coords = coordinates.reshape(-1, 3).astype(np.float32, copy=True)
    dummy = (species.reshape(-1) == -1)
    n_atoms = coords.shape[0]
    # Fold the dummy mask into per-side coordinate tables: distinct large
    # offsets per atom and per side so any dummy-involving pair exceeds the
    # cutoff; non-dummy rows stay bit-identical.
    off = (1e4 + 0.01 * np.arange(n_atoms, dtype=np.float64)).astype(np.float32)
    mod_a = coords.copy()
    mod_b = coords.copy()
    mod_a[dummy, 0] += off[dummy]
    mod_b[dummy, 0] -= off[dummy]

    i0 = neighbor_idxs[0].astype(np.int32)
    i1 = neighbor_idxs[1].astype(np.int32)
    ca_full = mod_a[i0]                     # [P, 3] f32
    cb_full = mod_b[i1]
    sh_full = shift_values.astype(np.float32, copy=False)

    nc = _build_kernel()
    in_maps = []
    for c in range(N_CORES):
        s = slice(c * P_CORE, (c + 1) * P_CORE)
        in_maps.append({
            "ca": ca_full[s].reshape(ROWS, COLS * 3),
            "cb": cb_full[s].reshape(ROWS, COLS * 3),
            "sh": sh_full[s].reshape(ROWS, COLS * 3),
            "i0": i0[s].reshape(ROWS, COLS),
            "i1": i1[s].reshape(ROWS, COLS),
        })
    res = bass_utils.run_bass_kernel_spmd(
        nc, in_maps, core_ids=list(range(N_CORES)))

    indices = np.empty((2, P_TOTAL), np.int32)
    distances = np.empty((P_TOTAL,), np.float32)
    diff_vectors = np.empty((P_TOTAL, 3), np.float32)
    for c in range(N_CORES):
        s = slice(c * P_CORE, (c + 1) * P_CORE)
        r = res.results[c]
        indices[0, s] = r["o_i0"].reshape(-1)
        indices[1, s] = r["o_i1"].reshape(-1)
        distances[s] = r["o_d"].reshape(-1)
        diff_vectors[s] = r["o_v"].reshape(-1, 3)
    return indices, distances, diff_vectors


# revision 4
# speedup vs baseline: 1.5417x; 1.5417x over previous
"""Trainium2 kernel for nn_Neighborlist (fused gather->diff->norm->screen).

Sharding: pairs are split evenly across the 8 NeuronCores (data parallel, per
the sharding hint). The host prepares per-core streams; each core computes
diff = cA - cB + shift, dist, and the cutoff/dummy screen, and writes the
masked outputs. The dummy-atom (species == -1) mask is folded into the
coordinate tables on the host: dummy atoms get a large per-atom offset (+ for
the i0 side, - for the i1 side) so that any pair involving a dummy atom
screens out via the cutoff test; non-dummy coordinates are bit-identical to
the originals so kept pairs are exact.

Note on the gather: the intended on-device gather from a replicated coords
table is not expressible on this toolchain — indirect_dma_start consumes only
one index per partition on HW, InstDMAGatherAnt (ext-isa dma_gather) wedges
the device (NRT_EXEC_UNIT_UNRECOVERABLE), and XLA's own gather lowering
crashes neuronx-cc on this module. The gather is therefore done host-side as
part of input sharding; the full screen/norm pipeline runs on device.

The cutoff test is done on squared distance against a precomputed threshold
DSQ_MAX = max{x : sqrt_f32(x) <= f32(5.2)}, which matches the reference's
sqrt-then-compare bit-exactly. dist itself is ACT Sqrt refined with one
Newton-Raphson step (rsqrt form) on the vector engine.
"""
import numpy as np

import concourse.bass as bass
import concourse.mybir as mybir
from concourse import bass_utils
from concourse.tile import TileContext
from concourse.vector_clock import ScopedClock
from concourse.alu_op_type import AluOpType

CUTOFF = np.float32(5.2)
N_CORES = 8
P_TOTAL = 6_400_000
P_CORE = P_TOTAL // N_CORES          # 800_000
ROWS = 128
COLS = P_CORE // ROWS                # 6250
F = 625                              # pairs per partition-row per tile
N_TILES = COLS // F                  # 10

_MODE_TO_OP = {
    "sem-ge-imm": "sem-ge",
    "sem-eq-imm": "sem-eq",
    "sem-le-imm": "sem-le",
    "sem-gt-imm": "sem-gt",
    "sem-lt-imm": "sem-lt",
}


def _patched_drain_and_barrier(self, tick_clock, wait_clock):
    # This walrus build encodes at most one sync wait per instruction and none
    # on TPB_CTRL (Drain/NoOp); emit the tail-drain waits as standalone
    # EventSemaphore instructions instead.
    import bass_rust
    nc = self.nc
    carrier = nc.sync.nop(nofuse=True, hint="pre_drain_waits")
    wait_clock.add_sem_waits(
        carrier.ins, ScopedClock({None: tick_clock.global_clock})
    )
    si = carrier.ins.sync_info
    waits = list(si.on_wait) if si is not None and si.on_wait else []
    if waits:
        si.on_wait = []
        for w in waits:
            handle = bass_rust.SemaphoreHandle(name=w.ant_name, num=w.id)
            nc.sync.wait_op(handle, w.wait_value, _MODE_TO_OP[w.wait_mode])
    nc.sync.drain()
    nc.all_engine_barrier()
    assert self.sems is not None
    popped = nc._tile_sem_poison_stack.pop()
    assert popped is self._sem_poison
    nc.clear_and_free_semaphores(list(self.sems.allocated().values()))
    nc.all_engine_barrier()


def _split_multi_waits(nc):
    # Hoist extra sync waits (>1 per instruction) into standalone
    # EventSemaphore instructions on the same engine.
    import bass_rust
    n = 0
    for f in nc.m.functions:
        for bb in f.blocks:
            out = []
            for inst in list(bb.instructions):
                si = inst.sync_info
                if si is not None and si.on_wait and len(si.on_wait) > 1:
                    extra = list(si.on_wait[:-1])
                    si.on_wait = si.on_wait[-1:]
                    for w in extra:
                        n += 1
                        ev = bass_rust.InstEventSemaphore(
                            name=f"I-msw-{n}-{inst.name}",
                            opcode="EventSemaphore",
                            engine=inst.engine,
                            debug=inst.debug,
                            ins=[],
                            outs=[],
                            sync_info=bass_rust.SyncInfo(on_wait=[w], on_update=[]),
                        )
                        out.append(ev)
                out.append(inst)
            bb.instructions = out
    return n


def _dsq_max():
    """Largest f32 x with sqrt_f32(x) <= f32(5.2)."""
    c = np.float32(5.2)
    x = np.float32(c * c)
    while np.sqrt(np.float32(x)) <= c:
        x = np.nextafter(x, np.float32(np.inf), dtype=np.float32)
    while np.sqrt(np.float32(x)) > c:
        x = np.nextafter(x, np.float32(-np.inf), dtype=np.float32)
    return float(x)


_NC_CACHE = {}


def _build_kernel():
    if "nc" in _NC_CACHE:
        return _NC_CACHE["nc"]
    from concourse.tile import TileContext as TC
    TC._drain_and_barrier = _patched_drain_and_barrier

    dsq_max = _dsq_max()
    f32 = mybir.dt.float32
    i32 = mybir.dt.int32
    nc = bass.Bass()
    ca = nc.dram_tensor("ca", [ROWS, COLS * 3], f32, kind="ExternalInput")
    cb = nc.dram_tensor("cb", [ROWS, COLS * 3], f32, kind="ExternalInput")
    sh = nc.dram_tensor("sh", [ROWS, COLS * 3], f32, kind="ExternalInput")
    i0 = nc.dram_tensor("i0", [ROWS, COLS], i32, kind="ExternalInput")
    i1 = nc.dram_tensor("i1", [ROWS, COLS], i32, kind="ExternalInput")
    o_i0 = nc.dram_tensor("o_i0", [ROWS, COLS], i32, kind="ExternalOutput")
    o_i1 = nc.dram_tensor("o_i1", [ROWS, COLS], i32, kind="ExternalOutput")
    o_d = nc.dram_tensor("o_d", [ROWS, COLS], f32, kind="ExternalOutput")
    o_v = nc.dram_tensor("o_v", [ROWS, COLS * 3], f32, kind="ExternalOutput")

    with TileContext(nc) as tc:
        with tc.tile_pool(name="p", bufs=2) as pool:
            for t in range(N_TILES):
                s3 = slice(t * F * 3, (t + 1) * F * 3)
                s1 = slice(t * F, (t + 1) * F)
                ca_t = pool.tile([ROWS, F * 3], f32, tag="ca")
                cb_t = pool.tile([ROWS, F * 3], f32, tag="cb")
                sh_t = pool.tile([ROWS, F * 3], f32, tag="sh")
                i0_t = pool.tile([ROWS, F], i32, tag="i0")
                i1_t = pool.tile([ROWS, F], i32, tag="i1")
                nc.sync.dma_start(ca_t[:], ca[:, s3])
                nc.sync.dma_start(cb_t[:], cb[:, s3])
                nc.sync.dma_start(sh_t[:], sh[:, s3])
                nc.sync.dma_start(i0_t[:], i0[:, s1])
                nc.sync.dma_start(i1_t[:], i1[:, s1])

                d3 = pool.tile([ROWS, F * 3], f32, tag="d3")
                nc.vector.tensor_tensor(
                    out=d3[:], in0=ca_t[:], in1=cb_t[:], op=AluOpType.subtract)
                nc.vector.tensor_tensor(
                    out=d3[:], in0=d3[:], in1=sh_t[:], op=AluOpType.add)
                sq = pool.tile([ROWS, F * 3], f32, tag="sq")
                nc.vector.tensor_tensor(
                    out=sq[:], in0=d3[:], in1=d3[:], op=AluOpType.mult)
                dsq = pool.tile([ROWS, F], f32, tag="dsq")
                nc.vector.reduce_sum(
                    dsq[:], sq[:].rearrange("p (k c) -> p k c", c=3),
                    axis=mybir.AxisListType.X)

                # dist = sqrt(dsq) : ACT Sqrt + one Newton step via exact DVE
                # reciprocal; dist=0 at dsq=0.
                d0 = pool.tile([ROWS, F], f32, tag="d0")
                nc.scalar.activation(
                    d0[:], dsq[:], mybir.ActivationFunctionType.Sqrt)
                dg = pool.tile([ROWS, F], f32, tag="dg")
                nc.vector.tensor_scalar(
                    out=dg[:], in0=d0[:], scalar1=1e-30, scalar2=None,
                    op0=AluOpType.max)
                inv = pool.tile([ROWS, F], f32, tag="inv")
                nc.vector.reciprocal(out=inv[:], in_=dg[:])
                y0 = pool.tile([ROWS, F], f32, tag="y0")
                nc.vector.tensor_tensor(
                    out=y0[:], in0=dsq[:], in1=inv[:], op=AluOpType.mult)
                dist = pool.tile([ROWS, F], f32, tag="dist")
                nc.vector.tensor_tensor(
                    out=dist[:], in0=d0[:], in1=y0[:], op=AluOpType.add)
                nc.vector.tensor_scalar(
                    out=dist[:], in0=dist[:], scalar1=0.5, scalar2=None,
                    op0=AluOpType.mult)

                keep = pool.tile([ROWS, F], f32, tag="keep")
                nc.vector.tensor_scalar(
                    out=keep[:], in0=dsq[:], scalar1=dsq_max, scalar2=None,
                    op0=AluOpType.is_le)

                od_t = pool.tile([ROWS, F], f32, tag="od")
                nc.vector.tensor_tensor(
                    out=od_t[:], in0=dist[:], in1=keep[:], op=AluOpType.mult)
                nc.sync.dma_start(o_d[:, s1], od_t[:])

                ov_t = pool.tile([ROWS, F * 3], f32, tag="ov")
                d3v = d3[:].rearrange("p (k c) -> p k c", c=3)
                ovv = ov_t[:].rearrange("p (k c) -> p k c", c=3)
                for c in range(3):
                    nc.vector.tensor_tensor(
                        out=ovv[:, :, c], in0=d3v[:, :, c], in1=keep[:],
                        op=AluOpType.mult)
                nc.sync.dma_start(o_v[:, s3], ov_t[:])

                for idx_t, o_t in ((i0_t, o_i0), (i1_t, o_i1)):
                    f_t = pool.tile([ROWS, F], f32, tag="fidx")
                    nc.vector.tensor_copy(out=f_t[:], in_=idx_t[:])
                    nc.vector.tensor_scalar(
                        out=f_t[:], in0=f_t[:], scalar1=1.0, scalar2=None,
                        op0=AluOpType.add)
                    nc.vector.tensor_tensor(
                        out=f_t[:], in0=f_t[:], in1=keep[:], op=AluOpType.mult)
                    nc.vector.tensor_scalar(
                        out=f_t[:], in0=f_t[:], scalar1=-1.0, scalar2=None,
                        op0=AluOpType.add)
                    oi_t = pool.tile([ROWS, F], i32, tag="oidx")
                    nc.vector.tensor_copy(out=oi_t[:], in_=f_t[:])
                    nc.sync.dma_start(o_t[:, s1], oi_t[:])

    _split_multi_waits(nc)
    _NC_CACHE["nc"] = nc
    return nc


def kernel(species, coordinates, neighbor_idxs, shift_values):
    species = np.asarray(species)
    coordinates = np.asarray(coordinates)
    neighbor_idxs = np.asarray(neighbor_idxs)
    shift_values = np.asarray(shift_values)

    coords = coordinates.reshape(-1, 3).astype(np.float32, copy=True)
    dummy = (species.reshape(-1) == -1)
    n_atoms = coords.shape[0]
    # Fold the dummy mask into per-side coordinate tables: distinct large
    # offsets per atom and per side so any dummy-involving pair exceeds the
    # cutoff; non-dummy rows stay bit-identical.
    off = (1e4 + 0.01 * np.arange(n_atoms, dtype=np.float64)).astype(np.float32)
    mod_a = coords.copy()
    mod_b = coords.copy()
    mod_a[dummy, 0] += off[dummy]
    mod_b[dummy, 0] -= off[dummy]

    i0 = neighbor_idxs[0].astype(np.int32)
    i1 = neighbor_idxs[1].astype(np.int32)
    ca_full = mod_a[i0]                     # [P, 3] f32
    cb_full = mod_b[i1]
    sh_full = shift_values.astype(np.float32, copy=False)

    nc = _build_kernel()
    in_maps = []
    for c in range(N_CORES):
        s = slice(c * P_CORE, (c + 1) * P_CORE)
        in_maps.append({
            "ca": ca_full[s].reshape(ROWS, COLS * 3),
            "cb": cb_full[s].reshape(ROWS, COLS * 3),
            "sh": sh_full[s].reshape(ROWS, COLS * 3),
            "i0": i0[s].reshape(ROWS, COLS),
            "i1": i1[s].reshape(ROWS, COLS),
        })
    res = bass_utils.run_bass_kernel_spmd(
        nc, in_maps, core_ids=list(range(N_CORES)))

    indices = np.empty((2, P_TOTAL), np.int32)
    distances = np.empty((P_TOTAL,), np.float32)
    diff_vectors = np.empty((P_TOTAL, 3), np.float32)
    for c in range(N_CORES):
        s = slice(c * P_CORE, (c + 1) * P_CORE)
        r = res.results[c]
        indices[0, s] = r["o_i0"].reshape(-1)
        indices[1, s] = r["o_i1"].reshape(-1)
        distances[s] = r["o_d"].reshape(-1)
        diff_vectors[s] = r["o_v"].reshape(-1, 3)
    # match the reference's output dtype (jnp.where preserves the input
    # neighbor_idxs dtype)
    if indices.dtype != neighbor_idxs.dtype:
        indices = indices.astype(neighbor_idxs.dtype)
    return indices, distances, diff_vectors


# revision 6
# speedup vs baseline: 1.6311x; 1.0580x over previous
"""Trainium2 kernel for nn_Neighborlist (fused gather->diff->norm->screen).

Sharding: pairs are split evenly across the 8 NeuronCores (data parallel, per
the sharding hint). The host prepares per-core streams; each core computes
diff = cA - cB + shift, dist, and the cutoff/dummy screen, and writes the
masked outputs. The dummy-atom (species == -1) mask is folded into the
coordinate tables on the host: dummy atoms get a large per-atom offset (+ for
the i0 side, - for the i1 side) so that any pair involving a dummy atom
screens out via the cutoff test; non-dummy coordinates are bit-identical to
the originals so kept pairs are exact.

Note on the gather: the intended on-device gather from a replicated coords
table is not expressible on this toolchain — indirect_dma_start consumes only
one index per partition on HW, InstDMAGatherAnt (ext-isa dma_gather) wedges
the device (NRT_EXEC_UNIT_UNRECOVERABLE), and XLA's own gather lowering
crashes neuronx-cc on this module. The gather is therefore done host-side as
part of input sharding; the full screen/norm pipeline runs on device.

The cutoff test is done on squared distance against a precomputed threshold
DSQ_MAX = max{x : sqrt_f32(x) <= f32(5.2)}, which matches the reference's
sqrt-then-compare bit-exactly. dist itself is ACT Sqrt refined with one
Newton-Raphson step (rsqrt form) on the vector engine.
"""
import numpy as np

import concourse.bass as bass
import concourse.mybir as mybir
from concourse import bass_utils
from concourse.tile import TileContext
from concourse.vector_clock import ScopedClock
from concourse.alu_op_type import AluOpType

CUTOFF = np.float32(5.2)
N_CORES = 8
P_TOTAL = 6_400_000
P_CORE = P_TOTAL // N_CORES          # 800_000
ROWS = 128
COLS = P_CORE // ROWS                # 6250
F = 625                              # pairs per partition-row per tile
N_TILES = COLS // F                  # 10

_MODE_TO_OP = {
    "sem-ge-imm": "sem-ge",
    "sem-eq-imm": "sem-eq",
    "sem-le-imm": "sem-le",
    "sem-gt-imm": "sem-gt",
    "sem-lt-imm": "sem-lt",
}


def _patched_drain_and_barrier(self, tick_clock, wait_clock):
    # This walrus build encodes at most one sync wait per instruction and none
    # on TPB_CTRL (Drain/NoOp); emit the tail-drain waits as standalone
    # EventSemaphore instructions instead.
    import bass_rust
    nc = self.nc
    carrier = nc.sync.nop(nofuse=True, hint="pre_drain_waits")
    wait_clock.add_sem_waits(
        carrier.ins, ScopedClock({None: tick_clock.global_clock})
    )
    si = carrier.ins.sync_info
    waits = list(si.on_wait) if si is not None and si.on_wait else []
    if waits:
        si.on_wait = []
        for w in waits:
            handle = bass_rust.SemaphoreHandle(name=w.ant_name, num=w.id)
            nc.sync.wait_op(handle, w.wait_value, _MODE_TO_OP[w.wait_mode])
    nc.sync.drain()
    nc.all_engine_barrier()
    assert self.sems is not None
    popped = nc._tile_sem_poison_stack.pop()
    assert popped is self._sem_poison
    nc.clear_and_free_semaphores(list(self.sems.allocated().values()))
    nc.all_engine_barrier()


def _split_multi_waits(nc):
    # Hoist extra sync waits (>1 per instruction) into standalone
    # EventSemaphore instructions on the same engine.
    import bass_rust
    n = 0
    for f in nc.m.functions:
        for bb in f.blocks:
            out = []
            for inst in list(bb.instructions):
                si = inst.sync_info
                if si is not None and si.on_wait and len(si.on_wait) > 1:
                    extra = list(si.on_wait[:-1])
                    si.on_wait = si.on_wait[-1:]
                    for w in extra:
                        n += 1
                        ev = bass_rust.InstEventSemaphore(
                            name=f"I-msw-{n}-{inst.name}",
                            opcode="EventSemaphore",
                            engine=inst.engine,
                            debug=inst.debug,
                            ins=[],
                            outs=[],
                            sync_info=bass_rust.SyncInfo(on_wait=[w], on_update=[]),
                        )
                        out.append(ev)
                out.append(inst)
            bb.instructions = out
    return n


def _dsq_max():
    """Largest f32 x with sqrt_f32(x) <= f32(5.2)."""
    c = np.float32(5.2)
    x = np.float32(c * c)
    while np.sqrt(np.float32(x)) <= c:
        x = np.nextafter(x, np.float32(np.inf), dtype=np.float32)
    while np.sqrt(np.float32(x)) > c:
        x = np.nextafter(x, np.float32(-np.inf), dtype=np.float32)
    return float(x)


_NC_CACHE = {}


def _build_kernel():
    if "nc" in _NC_CACHE:
        return _NC_CACHE["nc"]
    from concourse.tile import TileContext as TC
    TC._drain_and_barrier = _patched_drain_and_barrier

    dsq_max = _dsq_max()
    f32 = mybir.dt.float32
    i32 = mybir.dt.int32
    nc = bass.Bass()
    ca = nc.dram_tensor("ca", [ROWS, COLS * 3], f32, kind="ExternalInput")
    cb = nc.dram_tensor("cb", [ROWS, COLS * 3], f32, kind="ExternalInput")
    sh = nc.dram_tensor("sh", [ROWS, COLS * 3], f32, kind="ExternalInput")
    i0 = nc.dram_tensor("i0", [ROWS, COLS], i32, kind="ExternalInput")
    i1 = nc.dram_tensor("i1", [ROWS, COLS], i32, kind="ExternalInput")
    o_i0 = nc.dram_tensor("o_i0", [ROWS, COLS], i32, kind="ExternalOutput")
    o_i1 = nc.dram_tensor("o_i1", [ROWS, COLS], i32, kind="ExternalOutput")
    o_d = nc.dram_tensor("o_d", [ROWS, COLS], f32, kind="ExternalOutput")
    o_v = nc.dram_tensor("o_v", [ROWS, COLS * 3], f32, kind="ExternalOutput")

    with TileContext(nc) as tc:
        with tc.tile_pool(name="p", bufs=2) as pool:
            for t in range(N_TILES):
                s3 = slice(t * F * 3, (t + 1) * F * 3)
                s1 = slice(t * F, (t + 1) * F)
                ca_t = pool.tile([ROWS, F * 3], f32, tag="ca")
                cb_t = pool.tile([ROWS, F * 3], f32, tag="cb")
                sh_t = pool.tile([ROWS, F * 3], f32, tag="sh")
                i0_t = pool.tile([ROWS, F], i32, tag="i0")
                i1_t = pool.tile([ROWS, F], i32, tag="i1")
                nc.sync.dma_start(ca_t[:], ca[:, s3])
                nc.sync.dma_start(cb_t[:], cb[:, s3])
                nc.sync.dma_start(sh_t[:], sh[:, s3])
                nc.sync.dma_start(i0_t[:], i0[:, s1])
                nc.sync.dma_start(i1_t[:], i1[:, s1])

                d3 = pool.tile([ROWS, F * 3], f32, tag="d3")
                nc.vector.tensor_tensor(
                    out=d3[:], in0=ca_t[:], in1=cb_t[:], op=AluOpType.subtract)
                nc.vector.tensor_tensor(
                    out=d3[:], in0=d3[:], in1=sh_t[:], op=AluOpType.add)
                sq = pool.tile([ROWS, F * 3], f32, tag="sq")
                nc.vector.tensor_tensor(
                    out=sq[:], in0=d3[:], in1=d3[:], op=AluOpType.mult)
                dsq = pool.tile([ROWS, F], f32, tag="dsq")
                nc.vector.reduce_sum(
                    dsq[:], sq[:].rearrange("p (k c) -> p k c", c=3),
                    axis=mybir.AxisListType.X)

                # dist = sqrt(dsq) : ACT Sqrt + one Newton step via exact DVE
                # reciprocal; dist=0 at dsq=0.
                d0 = pool.tile([ROWS, F], f32, tag="d0")
                nc.scalar.activation(
                    d0[:], dsq[:], mybir.ActivationFunctionType.Sqrt)
                dg = pool.tile([ROWS, F], f32, tag="dg")
                nc.vector.tensor_scalar(
                    out=dg[:], in0=d0[:], scalar1=1e-30, scalar2=None,
                    op0=AluOpType.max)
                inv = pool.tile([ROWS, F], f32, tag="inv")
                nc.vector.reciprocal(out=inv[:], in_=dg[:])
                y0 = pool.tile([ROWS, F], f32, tag="y0")
                nc.vector.tensor_tensor(
                    out=y0[:], in0=dsq[:], in1=inv[:], op=AluOpType.mult)
                dist = pool.tile([ROWS, F], f32, tag="dist")
                nc.vector.tensor_tensor(
                    out=dist[:], in0=d0[:], in1=y0[:], op=AluOpType.add)
                nc.vector.tensor_scalar(
                    out=dist[:], in0=dist[:], scalar1=0.5, scalar2=None,
                    op0=AluOpType.mult)

                keep = pool.tile([ROWS, F], f32, tag="keep")
                nc.vector.tensor_scalar(
                    out=keep[:], in0=dsq[:], scalar1=dsq_max, scalar2=None,
                    op0=AluOpType.is_le)

                od_t = pool.tile([ROWS, F], f32, tag="od")
                nc.vector.tensor_tensor(
                    out=od_t[:], in0=dist[:], in1=keep[:], op=AluOpType.mult)
                nc.sync.dma_start(o_d[:, s1], od_t[:])

                ov_t = pool.tile([ROWS, F * 3], f32, tag="ov")
                d3v = d3[:].rearrange("p (k c) -> p k c", c=3)
                ovv = ov_t[:].rearrange("p (k c) -> p k c", c=3)
                for c in range(3):
                    nc.vector.tensor_tensor(
                        out=ovv[:, :, c], in0=d3v[:, :, c], in1=keep[:],
                        op=AluOpType.mult)
                nc.sync.dma_start(o_v[:, s3], ov_t[:])

                for idx_t, o_t in ((i0_t, o_i0), (i1_t, o_i1)):
                    # (idx+1)*keep - 1 == where(keep, idx, -1); the int32<->f32
                    # conversions ride the tensor_scalar ops.
                    f_t = pool.tile([ROWS, F], f32, tag="fidx")
                    nc.vector.tensor_scalar(
                        out=f_t[:], in0=idx_t[:], scalar1=1.0, scalar2=None,
                        op0=AluOpType.add)
                    nc.vector.tensor_tensor(
                        out=f_t[:], in0=f_t[:], in1=keep[:], op=AluOpType.mult)
                    oi_t = pool.tile([ROWS, F], i32, tag="oidx")
                    nc.vector.tensor_scalar(
                        out=oi_t[:], in0=f_t[:], scalar1=-1.0, scalar2=None,
                        op0=AluOpType.add)
                    nc.sync.dma_start(o_t[:, s1], oi_t[:])

    _split_multi_waits(nc)
    _NC_CACHE["nc"] = nc
    return nc


def kernel(species, coordinates, neighbor_idxs, shift_values):
    species = np.asarray(species)
    coordinates = np.asarray(coordinates)
    neighbor_idxs = np.asarray(neighbor_idxs)
    shift_values = np.asarray(shift_values)

    coords = coordinates.reshape(-1, 3).astype(np.float32, copy=True)
    dummy = (species.reshape(-1) == -1)
    n_atoms = coords.shape[0]
    # Fold the dummy mask into per-side coordinate tables: distinct large
    # offsets per atom and per side so any dummy-involving pair exceeds the
    # cutoff; non-dummy rows stay bit-identical.
    off = (1e4 + 0.01 * np.arange(n_atoms, dtype=np.float64)).astype(np.float32)
    mod_a = coords.copy()
    mod_b = coords.copy()
    mod_a[dummy, 0] += off[dummy]
    mod_b[dummy, 0] -= off[dummy]

    i0 = neighbor_idxs[0].astype(np.int32)
    i1 = neighbor_idxs[1].astype(np.int32)
    ca_full = mod_a[i0]                     # [P, 3] f32
    cb_full = mod_b[i1]
    sh_full = shift_values.astype(np.float32, copy=False)

    nc = _build_kernel()
    in_maps = []
    for c in range(N_CORES):
        s = slice(c * P_CORE, (c + 1) * P_CORE)
        in_maps.append({
            "ca": ca_full[s].reshape(ROWS, COLS * 3),
            "cb": cb_full[s].reshape(ROWS, COLS * 3),
            "sh": sh_full[s].reshape(ROWS, COLS * 3),
            "i0": i0[s].reshape(ROWS, COLS),
            "i1": i1[s].reshape(ROWS, COLS),
        })
    res = bass_utils.run_bass_kernel_spmd(
        nc, in_maps, core_ids=list(range(N_CORES)))

    indices = np.empty((2, P_TOTAL), np.int32)
    distances = np.empty((P_TOTAL,), np.float32)
    diff_vectors = np.empty((P_TOTAL, 3), np.float32)
    for c in range(N_CORES):
        s = slice(c * P_CORE, (c + 1) * P_CORE)
        r = res.results[c]
        indices[0, s] = r["o_i0"].reshape(-1)
        indices[1, s] = r["o_i1"].reshape(-1)
        distances[s] = r["o_d"].reshape(-1)
        diff_vectors[s] = r["o_v"].reshape(-1, 3)
    # match the reference's output dtype (jnp.where preserves the input
    # neighbor_idxs dtype)
    if indices.dtype != neighbor_idxs.dtype:
        indices = indices.astype(neighbor_idxs.dtype)
    return indices, distances, diff_vectors
